# revision 2
# baseline (speedup 1.0000x reference)
"""RNN kernel for Trainium2: h_t = tanh(x_t @ Wx.T + h_{t-1} @ Wh.T + b).

Sharding: batch B=256 split across 8 cores (32 each); weights replicated.
Per-core layout: everything lives in "transposed" (feature-on-partition)
form. Phase A computes P = Wx @ x_t.T for 16 timesteps directly into the
8 PSUM banks (one bank per 128-dim output group, 512 cols = 16 steps x 32
batch). Phase B accumulates Wh @ h_{t-1}.T on top (start=False), then the
ACT engine applies tanh(+bias) producing the fp32 output staging tile and
the DVE makes the bf16 h for the next step's matmuls.
"""

import os
import sys
import types

import numpy as np
import ml_dtypes

import concourse.mybir as mybir

try:
    from antenv.axon_hooks import get_axon_ntff_profile_hook  # noqa: F401
except ImportError:
    from trn_agent_boot.trn_boot import _ntff_profile_via_ctypes

    _hook = _ntff_profile_via_ctypes("/opt/axon/libaxon_pjrt.so")
    _mod = types.ModuleType("antenv.axon_hooks")
    _mod.get_axon_ntff_profile_hook = lambda: _hook
    sys.modules["antenv.axon_hooks"] = _mod

from concourse import bacc, bass, tile
from concourse.bass_utils import run_bass_kernel_spmd

FP32 = mybir.dt.float32
BF16 = mybir.dt.float16  # fp16: same PE speed as bf16, 3 more mantissa bits
AF = mybir.ActivationFunctionType

T, B, D = 256, 256, 1024
NCORES = 8
BL = B // NCORES          # 32 batch per core
HB = 16                   # timesteps per half-block (16*32 = 512 moving cols)
NHB = T // HB             # 16 half-blocks
COLS = HB * BL            # 512

_cached = {}


def _build():
    if "nc" in _cached:
        return _cached["nc"]

    nc = bacc.Bacc(None, target_bir_lowering=False, debug=True)
    wx_in = nc.dram_tensor("wx", [128, 64 * 128], BF16, kind="ExternalInput")
    wh_in = nc.dram_tensor("wh", [128, 64 * 128], BF16, kind="ExternalInput")
    bias_in = nc.dram_tensor("bias", [128, 8], FP32, kind="ExternalInput")
    xt_in = nc.dram_tensor("xt", [128, 8 * T * BL], BF16, kind="ExternalInput")
    out_dr = nc.dram_tensor("out", [128, 8 * T * BL], FP32, kind="ExternalOutput")

    with tile.TileContext(nc) as tc:
        with tc.tile_pool(name="sbuf", bufs=1) as pool, \
             tc.tile_pool(name="psum", bufs=1, space=bass.MemorySpace.PSUM) as ppool:
            wx_sb = pool.tile([128, 64 * 128], BF16)
            wh_sb = pool.tile([128, 64 * 128], BF16)
            bias_sb = pool.tile([128, 8], FP32)
            nc.gpsimd.dma_start(out=wx_sb[:], in_=wx_in[:])
            nc.gpsimd.dma_start(out=wh_sb[:], in_=wh_in[:])
            nc.gpsimd.dma_start(out=bias_sb[:], in_=bias_in[:])

            xb0 = pool.tile([128, 8 * COLS], BF16)
            xb1 = pool.tile([128, 8 * COLS], BF16)
            xbufs = [xb0, xb1]

            ob0 = [pool.tile([128, COLS], FP32, name=f"ob0_{j}") for j in range(8)]
            ob1 = [pool.tile([128, COLS], FP32, name=f"ob1_{j}") for j in range(8)]
            ob2 = [pool.tile([128, COLS], FP32, name=f"ob2_{j}") for j in range(8)]
            obufs = [ob0, ob1, ob2]

            hA = pool.tile([128, 8 * BL], BF16)
            hB = pool.tile([128, 8 * BL], BF16)
            nc.vector.memset(hA[:], 0.0)

            ps = [ppool.tile([128, COLS], FP32, name=f"ps{j}") for j in range(8)]

            def x_dma(hb):
                xb = xbufs[hb % 2]
                for kg in range(8):
                    nc.gpsimd.dma_start(
                        out=xb[:, kg * COLS:(kg + 1) * COLS],
                        in_=xt_in[:, kg * T * BL + hb * COLS: kg * T * BL + (hb + 1) * COLS],
                    )

            x_dma(0)
            cur, nxt = hA, hB
            for hb in range(NHB):
                xb = xbufs[hb % 2]
                # Phase A: P = Wx @ xT for HB steps, straight into psum banks
                for jg in range(8):
                    for kg in range(8):
                        nc.tensor.matmul(
                            ps[jg][:, :],
                            wx_sb[:, (kg * 8 + jg) * 128:(kg * 8 + jg + 1) * 128],
                            xb[:, kg * COLS:(kg + 1) * COLS],
                            start=(kg == 0),
                            stop=(kg == 7),
                        )
                if hb + 1 < NHB:
                    x_dma(hb + 1)
                # Phase B: recurrence
                ob = obufs[hb % 3]
                for lt in range(HB):
                    for jg in range(8):
                        for kg in range(8):
                            nc.tensor.matmul(
                                ps[jg][:, lt * BL:(lt + 1) * BL],
                                wh_sb[:, (kg * 8 + jg) * 128:(kg * 8 + jg + 1) * 128],
                                cur[:, kg * BL:(kg + 1) * BL],
                                start=False,
                                stop=(kg == 7),
                                skip_group_check=True,
                            )
                        nc.scalar.activation(
                            out=ob[jg][:, lt * BL:(lt + 1) * BL],
                            in_=ps[jg][:, lt * BL:(lt + 1) * BL],
                            func=AF.Tanh,
                            bias=bias_sb[:, jg:jg + 1],
                            scale=1.0,
                        )
                        nc.vector.tensor_copy(
                            out=nxt[:, jg * BL:(jg + 1) * BL],
                            in_=ob[jg][:, lt * BL:(lt + 1) * BL],
                        )
                    cur, nxt = nxt, cur
                for jg in range(8):
                    nc.sync.dma_start(
                        out=out_dr[:, jg * T * BL + hb * COLS: jg * T * BL + (hb + 1) * COLS],
                        in_=ob[jg][:, :],
                    )

    nc.compile()
    _cached["nc"] = nc
    return nc


def kernel(x: np.ndarray, W: np.ndarray, b: np.ndarray) -> np.ndarray:
    nc = _build()

    Wx = W[:, :D]
    Wh = W[:, D:]
    # wx_np[p, (kg*8+jg)*128+q] = Wx[jg*128+q, kg*128+p]
    wx_np = np.ascontiguousarray(
        Wx.reshape(8, 128, 8, 128).transpose(3, 2, 0, 1).reshape(128, 64 * 128)
    ).astype(np.float16)
    wh_np = np.ascontiguousarray(
        Wh.reshape(8, 128, 8, 128).transpose(3, 2, 0, 1).reshape(128, 64 * 128)
    ).astype(np.float16)
    bias_np = np.ascontiguousarray(b.reshape(8, 128).T).astype(np.float32)

    ins = []
    for c in range(NCORES):
        xc = x[:, c * BL:(c + 1) * BL, :]                      # [T, BL, D]
        xT = xc.reshape(T * BL, D).T                           # [D, T*BL]
        xt_np = np.ascontiguousarray(
            xT.reshape(8, 128, T * BL).transpose(1, 0, 2).reshape(128, 8 * T * BL)
        ).astype(np.float16)
        ins.append({"wx": wx_np, "wh": wh_np, "bias": bias_np, "xt": xt_np})

    trace = bool(os.environ.get("BASS_KERNEL_TRACE"))
    res = run_bass_kernel_spmd(nc, ins, list(range(NCORES)), trace=trace)
    if trace:
        _cached["exec_time_ns"] = res.exec_time_ns

    out = np.empty((B, T, D), np.float32)
    for c in range(NCORES):
        oc = np.asarray(res.results[c]["out"])                 # [128, 8*T*BL]
        # oc[p, jg*T*BL + t*BL + b] = h_t[jg*128+p, b]
        out[c * BL:(c + 1) * BL] = (
            oc.reshape(128, 8, T, BL).transpose(3, 2, 1, 0).reshape(BL, T, D)
        )
    return out


if __name__ == "__main__":
    rng = np.random.default_rng(0)
    x = rng.standard_normal((T, B, D)).astype(np.float32)
    W = ((rng.uniform(-1, 1, (D, 2 * D))) / np.sqrt(2 * D)).astype(np.float32)
    b = ((rng.uniform(-1, 1, D)) / np.sqrt(2 * D)).astype(np.float32)
    got = kernel(x, W, b)
    # numpy reference
    Wx, Wh = W[:, :D], W[:, D:]
    h = np.zeros((B, D), np.float32)
    ref = np.empty((B, T, D), np.float32)
    for t in range(T):
        h = np.tanh(x[t] @ Wx.T + h @ Wh.T + b)
        ref[:, t, :] = h
    err = np.abs(got - ref).max() / np.abs(ref).max()
    print("self-check rel err:", err)


# revision 3
# speedup vs baseline: 1.3925x; 1.3925x over previous
"""RNN kernel for Trainium2: h_t = tanh(x_t @ Wx.T + h_{t-1} @ Wh.T + b).

Sharding: batch B=256 split across 8 cores (32 each); weights replicated.
Per-core layout: feature-on-partition ("transposed") form everywhere.
Phase A per 16-step block: bias (rank-1 matmul) + Wx @ x_t.T accumulate
into one 8-bank psum tile. Phase B per step: Wh @ h_{t-1}.T accumulates
on top (start=False); activations are fused across 4 feature groups per
instruction (bias already in psum) so the ACT chain stays off the
critical path, and matmuls are issued kg-major grouped by output half so
the next step's matmuls only wait on the matching half of h.
"""

import os
import sys
import types

import numpy as np

import concourse.mybir as mybir

try:
    from antenv.axon_hooks import get_axon_ntff_profile_hook  # noqa: F401
except ImportError:
    from trn_agent_boot.trn_boot import _ntff_profile_via_ctypes

    _hook = _ntff_profile_via_ctypes("/opt/axon/libaxon_pjrt.so")
    _mod = types.ModuleType("antenv.axon_hooks")
    _mod.get_axon_ntff_profile_hook = lambda: _hook
    sys.modules["antenv.axon_hooks"] = _mod

from concourse import bacc, bass, tile
from concourse.bass_utils import run_bass_kernel_spmd

FP32 = mybir.dt.float32
F16 = mybir.dt.float16  # same PE speed as bf16, 3 more mantissa bits
AF = mybir.ActivationFunctionType

T, B, D = 256, 256, 1024
NCORES = 8
BL = B // NCORES          # 32 batch per core
HB = 16                   # timesteps per block (16*32 = 512 psum cols/bank)
NHB = T // HB             # 16 blocks
COLS = HB * BL            # 512

_cached = {}


def _build():
    if "nc" in _cached:
        return _cached["nc"]

    nc = bacc.Bacc(None, target_bir_lowering=False, debug=True)
    wx_in = nc.dram_tensor("wx", [128, 64 * 128], F16, kind="ExternalInput")
    wh_in = nc.dram_tensor("wh", [128, 64 * 128], F16, kind="ExternalInput")
    b16_in = nc.dram_tensor("bias16", [1, D], F16, kind="ExternalInput")
    xt_in = nc.dram_tensor("xt", [128, 8 * T * BL], F16, kind="ExternalInput")
    out_dr = nc.dram_tensor("out", [128, 8 * T * BL], FP32, kind="ExternalOutput")

    with tile.TileContext(nc) as tc:
        with tc.tile_pool(name="sbuf", bufs=1) as pool, \
             tc.tile_pool(name="psum", bufs=1, space=bass.MemorySpace.PSUM) as ppool:
            wx_sb = pool.tile([128, 64 * 128], F16)
            wh_sb = pool.tile([128, 64 * 128], F16)
            b16_sb = pool.tile([1, D], F16)
            ones_sb = pool.tile([1, COLS], F16)
            nc.gpsimd.dma_start(out=wx_sb[:], in_=wx_in[:])
            nc.gpsimd.dma_start(out=wh_sb[:], in_=wh_in[:])
            nc.gpsimd.dma_start(out=b16_sb[:], in_=b16_in[:])
            nc.vector.memset(ones_sb[:], 1.0)

            xb0 = pool.tile([128, 8 * COLS], F16)
            xb1 = pool.tile([128, 8 * COLS], F16)
            xbufs = [xb0, xb1]

            obufs = [pool.tile([128, HB * 256], FP32, name=f"ob{s}") for s in range(3)]

            hA = pool.tile([128, 8 * BL], F16)
            hB = pool.tile([128, 8 * BL], F16)
            nc.vector.memset(hA[:], 0.0)

            ps = ppool.tile([128, 4096], FP32)   # all 8 banks; bank jg = cols jg*512
            ps3 = ps[:].rearrange("p (j c) -> p j c", j=8)

            def x_dma(hb):
                xb = xbufs[hb % 2]
                for kg in range(8):
                    nc.gpsimd.dma_start(
                        out=xb[:, kg * COLS:(kg + 1) * COLS],
                        in_=xt_in[:, kg * T * BL + hb * COLS: kg * T * BL + (hb + 1) * COLS],
                    )

            x_dma(0)
            cur, nxt = hA, hB
            for hb in range(NHB):
                xb = xbufs[hb % 2]
                # Phase A: psum bank jg := bias (rank-1) + Wx @ xT for HB steps
                for jg in range(8):
                    nc.tensor.matmul(
                        ps[:, jg * 512:(jg + 1) * 512],
                        b16_sb[0:1, jg * 128:(jg + 1) * 128],
                        ones_sb[0:1, :],
                        start=True,
                        stop=False,
                    )
                    for kg in range(8):
                        nc.tensor.matmul(
                            ps[:, jg * 512:(jg + 1) * 512],
                            wx_sb[:, (kg * 8 + jg) * 128:(kg * 8 + jg + 1) * 128],
                            xb[:, kg * COLS:(kg + 1) * COLS],
                            start=False,
                            stop=(kg == 7),
                        )
                if hb + 1 < NHB:
                    x_dma(hb + 1)
                # Phase B: recurrence
                ob = obufs[hb % 3]
                for lt in range(HB):
                    for half in range(2):
                        for kg in range(8):
                            for jg in range(half * 4, half * 4 + 4):
                                nc.tensor.matmul(
                                    ps[:, jg * 512 + lt * BL: jg * 512 + (lt + 1) * BL],
                                    wh_sb[:, (kg * 8 + jg) * 128:(kg * 8 + jg + 1) * 128],
                                    cur[:, kg * BL:(kg + 1) * BL],
                                    start=False,
                                    stop=(kg == 7),
                                    skip_group_check=True,
                                )
                        ob2 = ob[:, lt * 256:(lt + 1) * 256].rearrange(
                            "p (j c) -> p j c", j=8)
                        nc.scalar.activation(
                            out=ob2[:, half * 4:(half + 1) * 4, :],
                            in_=ps3[:, half * 4:(half + 1) * 4, lt * BL:(lt + 1) * BL],
                            func=AF.Tanh,
                            bias=0.0,
                            scale=1.0,
                        )
                        nc.vector.tensor_copy(
                            out=nxt[:, half * 128:(half + 1) * 128],
                            in_=ob[:, lt * 256 + half * 128: lt * 256 + (half + 1) * 128],
                        )
                    cur, nxt = nxt, cur
                nc.sync.dma_start(
                    out=out_dr[:, hb * HB * 256:(hb + 1) * HB * 256],
                    in_=ob[:, :],
                )

    nc.compile()
    _cached["nc"] = nc
    return nc


def kernel(x: np.ndarray, W: np.ndarray, b: np.ndarray) -> np.ndarray:
    nc = _build()

    Wx = W[:, :D]
    Wh = W[:, D:]
    # wx_np[p, (kg*8+jg)*128+q] = Wx[jg*128+q, kg*128+p]
    wx_np = np.ascontiguousarray(
        Wx.reshape(8, 128, 8, 128).transpose(3, 2, 0, 1).reshape(128, 64 * 128)
    ).astype(np.float16)
    wh_np = np.ascontiguousarray(
        Wh.reshape(8, 128, 8, 128).transpose(3, 2, 0, 1).reshape(128, 64 * 128)
    ).astype(np.float16)
    b16_np = b.reshape(1, D).astype(np.float16)

    ins = []
    for c in range(NCORES):
        xc = x[:, c * BL:(c + 1) * BL, :]                      # [T, BL, D]
        xT = xc.reshape(T * BL, D).T                           # [D, T*BL]
        xt_np = np.ascontiguousarray(
            xT.reshape(8, 128, T * BL).transpose(1, 0, 2).reshape(128, 8 * T * BL)
        ).astype(np.float16)
        ins.append({"wx": wx_np, "wh": wh_np, "bias16": b16_np, "xt": xt_np})

    trace = bool(os.environ.get("BASS_KERNEL_TRACE"))
    res = run_bass_kernel_spmd(nc, ins, list(range(NCORES)), trace=trace)
    if trace:
        _cached["exec_time_ns"] = res.exec_time_ns

    out = np.empty((B, T, D), np.float32)
    for c in range(NCORES):
        oc = np.asarray(res.results[c]["out"])                 # [128, 65536]
        # oc[p, hb*4096 + lt*256 + jg*32 + b] = h_{hb*16+lt}[jg*128+p, b]
        out[c * BL:(c + 1) * BL] = (
            oc.reshape(128, NHB, HB, 8, BL)
            .transpose(4, 1, 2, 3, 0)
            .reshape(BL, T, D)
        )
    return out


if __name__ == "__main__":
    rng = np.random.default_rng(0)
    x = rng.standard_normal((T, B, D)).astype(np.float32)
    W = ((rng.uniform(-1, 1, (D, 2 * D))) / np.sqrt(2 * D)).astype(np.float32)
    b = ((rng.uniform(-1, 1, D)) / np.sqrt(2 * D)).astype(np.float32)
    got = kernel(x, W, b)
    if "exec_time_ns" in _cached:
        print("HW exec time:", _cached["exec_time_ns"], "ns")
    Wx, Wh = W[:, :D], W[:, D:]
    h = np.zeros((B, D), np.float32)
    ref = np.empty((B, T, D), np.float32)
    for t in range(T):
        h = np.tanh(x[t] @ Wx.T + h @ Wh.T + b)
        ref[:, t, :] = h
    err = np.abs(got - ref).max() / np.abs(ref).max()
    print("self-check rel err:", err)
